# revision 1
# baseline (speedup 1.0000x reference)
"""Two-layer GAT encoder on 8 Trainium2 NeuronCores (Bass/Tile).

Strategy (graph/data parallel, dst-partitioned):
  - Nodes are partitioned contiguously across the 8 cores (6250 each); every
    edge lives on the core that owns its destination node, so softmax
    segments never cross cores and no boundary all-reduce is needed.
  - Each core computes the full layer-1 gather table (h1+b1 | alpha_src) for
    all 50000 nodes from a host-transposed copy of x (replicated compute,
    avoids any layer-1 collective), plus the alpha_dst table.
  - Edge phase per 128-node window: indirect-DMA row gathers of source rows
    and per-edge alpha_dst (one offset per partition row — the form the
    hardware DGE unroll supports), one-hot segment matrix via iota/is_equal,
    exp-weighted features + softmax denominator aggregated into one PSUM
    tile by a chain of matmuls, deferred division.
  - Layer-2 tables (h2+b2 | alpha_src2, alpha_dst2) are computed per
    partition from the layer-1 output and replicated with AllGathers.
  - Softmax runs without max-subtraction: |alpha_s + alpha_d| < ~8 for this
    architecture (weights scaled by 0.1), so exp() is safe in fp32.
"""

from contextlib import ExitStack

import numpy as np

import concourse.bass as bass
import concourse.bacc as bacc
import concourse.tile as tile
import concourse.mybir as mybir
from concourse.bass import IndirectOffsetOnAxis
from concourse.bass_utils import run_bass_kernel_spmd
from concourse.masks import make_identity

F32 = mybir.dt.float32
I32 = mybir.dt.int32

N = 50000
NCORES = 8
NPC = N // NCORES          # 6250 nodes per core
WIN = 128                  # nodes per window (= psum partition rows)
NWIN = (NPC + WIN - 1) // WIN   # 49
LASTW = NPC - (NWIN - 1) * WIN  # 106 valid rows in the final window
P = 128                    # edges per block
F1, H1, C1 = 128, 4, 32
F2, H2, C2 = 32, 1, 32
T1C = F1 + H1              # 132 table-1 row: [h1+b1 | alpha_s1]
T2C = F2 + H2              # 33  table-2 row: [h2+b2 | alpha_s2]
NPAD = 50048               # table rows padded to a multiple of 128

XGRP = 2048                # node columns fetched per table-build group


def _build_structures(edge_index):
    """Partition/sort edges by destination; pack into window-aligned blocks of
    128 edge slots. Returns per-core (srcI, dstI, relX) arrays of shape
    [128, NWIN*NB] plus NB (blocks per window, uniform across cores)."""
    src = np.concatenate([edge_index[0], np.arange(N, dtype=np.int64)]).astype(np.int32)
    dst = np.concatenate([edge_index[1], np.arange(N, dtype=np.int64)]).astype(np.int32)
    per_core = []
    nb_max = 1
    for c in range(NCORES):
        lo = c * NPC
        m = (dst >= lo) & (dst < lo + NPC)
        s_c = src[m]
        d_c = dst[m] - lo
        order = np.argsort(d_c, kind='stable')
        s_c, d_c = s_c[order], d_c[order]
        counts = np.bincount(d_c // WIN, minlength=NWIN)
        nb_max = max(nb_max, int(((counts + P - 1) // P).max()))
        per_core.append((s_c, d_c, counts, lo))
    NB = nb_max
    out = []
    for s_c, d_c, counts, lo in per_core:
        srcI = np.zeros((NWIN, NB * P), dtype=np.int32)
        dstI = np.zeros((NWIN, NB * P), dtype=np.int32)
        relX = np.full((NWIN, NB * P), 200.0, dtype=np.float32)
        starts = np.concatenate([[0], np.cumsum(counts)])
        for w in range(NWIN):
            cnt = int(counts[w])
            sl = slice(starts[w], starts[w] + cnt)
            srcI[w, :cnt] = s_c[sl]
            dstI[w, :cnt] = d_c[sl] + lo          # global id into [N,*] tables
            relX[w, :cnt] = (d_c[sl] - w * WIN).astype(np.float32)
        # slot (w, b, p) -> partition p, column w*NB+b
        def lay(a, dt):
            return np.ascontiguousarray(
                a.reshape(NWIN, NB, P).transpose(2, 0, 1).reshape(P, NWIN * NB)
            ).astype(dt)
        out.append((lay(srcI, np.int32), lay(dstI, np.int32), lay(relX, np.float32)))
    return out, NB


def gat_program(ctx, tc, ins, outs, NB):
    """Emit the full two-layer GAT program. `ins`/`outs` are dicts of DRAM APs:
    ins: xT [128,N], w1ext [128,136], w2ext [128,34], bias1B [128,136],
         bias2B [128,34], srcI/dstI [128,NWIN*NB] i32, relX [128,NWIN*NB] f32
    outs: out [NPC, 32]"""
    nc = tc.nc

    table1 = nc.dram_tensor("table1", [NPAD, T1C], F32)
    ad1 = nc.dram_tensor("ad1", [NPAD, H1], F32)
    t2loc = nc.dram_tensor("t2loc", [NPC, T2C], F32)
    ad2loc = nc.dram_tensor("ad2loc", [NPC, H2], F32)
    table2 = nc.dram_tensor("table2", [N, T2C], F32, addr_space="Shared")
    ad2 = nc.dram_tensor("ad2", [N, H2], F32, addr_space="Shared")

    const = ctx.enter_context(tc.tile_pool(name="const", bufs=1))
    p0 = ctx.enter_context(tc.tile_pool(name="p0", bufs=2))
    p0ps = ctx.enter_context(tc.tile_pool(name="p0ps", bufs=2, space="PSUM"))
    gp = ctx.enter_context(tc.tile_pool(name="gp", bufs=2))
    sm = ctx.enter_context(tc.tile_pool(name="sm", bufs=2))
    ohp = ctx.enter_context(tc.tile_pool(name="ohp", bufs=2))
    wp = ctx.enter_context(tc.tile_pool(name="wp", bufs=2))
    pw = ctx.enter_context(tc.tile_pool(name="pw", bufs=2, space="PSUM"))
    pt = ctx.enter_context(tc.tile_pool(name="pt", bufs=1, space="PSUM"))
    p2 = ctx.enter_context(tc.tile_pool(name="p2", bufs=1, space="PSUM"))

    # --- resident constants -------------------------------------------------
    w1t = const.tile([128, F1 + 2 * H1], F32)
    nc.sync.dma_start(out=w1t[:], in_=ins['w1ext'])
    w2t = const.tile([128, F2 + 2 * H2], F32)
    nc.sync.dma_start(out=w2t[:], in_=ins['w2ext'])
    b1t = const.tile([128, F1 + 2 * H1], F32)
    nc.sync.dma_start(out=b1t[:], in_=ins['bias1B'])
    b2t = const.tile([128, F2 + 2 * H2], F32)
    nc.sync.dma_start(out=b2t[:], in_=ins['bias2B'])
    srcit = const.tile([128, NWIN * NB], I32)
    nc.sync.dma_start(out=srcit[:], in_=ins['srcI'])
    dstit = const.tile([128, NWIN * NB], I32)
    nc.sync.dma_start(out=dstit[:], in_=ins['dstI'])
    relt = const.tile([128, NWIN * NB], F32)
    nc.sync.dma_start(out=relt[:], in_=ins['relX'])
    ident = const.tile([128, 128], F32)
    make_identity(nc, ident[:])
    iota_i = const.tile([128, NB * 128], I32)
    nc.gpsimd.iota(iota_i[:], pattern=[[0, NB], [1, 128]], base=0, channel_multiplier=0)
    iotaf = const.tile([128, NB * 128], F32)
    nc.vector.tensor_copy(out=iotaf[:], in_=iota_i[:])

    # --- phase 0: build table1 [h1+b1 | as1] and ad1 for all nodes ----------
    for gs in range(0, N, XGRP):
        gcols = min(XGRP, N - gs)
        nchunks = (gcols + 127) // 128
        xg = p0.tile([128, XGRP], F32, tag="xg")
        nc.sync.dma_start(out=xg[:, :gcols], in_=ins['xT'][:, gs:gs + gcols])
        t1g = p0.tile([128, XGRP // 128, F1 + 2 * H1], F32, tag="t1g")
        if gcols < XGRP:
            nc.vector.memset(t1g[:], 0.0)
        for cchunk in range(nchunks):
            m = min(128, gcols - cchunk * 128)
            ps = p0ps.tile([128, F1 + 2 * H1], F32)
            nc.tensor.matmul(
                out=ps[:m, :],
                lhsT=xg[:, cchunk * 128:cchunk * 128 + m],
                rhs=w1t[:],
                start=True, stop=True,
            )
            nc.vector.tensor_tensor(
                out=t1g[:m, cchunk, :], in0=ps[:m, :], in1=b1t[:m, :],
                op=mybir.AluOpType.add,
            )
        rows = nchunks * 128
        nc.sync.dma_start(
            out=table1.ap()[gs:gs + rows, :].rearrange("(c p) f -> p c f", p=128),
            in_=t1g[:, :nchunks, 0:T1C],
        )
        nc.sync.dma_start(
            out=ad1.ap()[gs:gs + rows, :].rearrange("(c p) f -> p c f", p=128),
            in_=t1g[:, :nchunks, T1C:T1C + H1],
        )

    # --- edge phase ---------------------------------------------------------
    def edge_layer(table_ap, ad_ap, FD, HD, sink):
        """FD feature cols, HD heads; sink(w, nrows, res_ap) consumes the
        [128, FD] per-window output (res rows beyond nrows are zeroed for the
        final window)."""
        TC = FD + HD
        for w in range(NWIN):
            nrows = LASTW if w == NWIN - 1 else 128
            g = gp.tile([128, NB, TC], F32, tag="g")
            a = sm.tile([128, NB, HD], F32, tag="a")
            for b in range(NB):
                col = w * NB + b
                # multi-offset indirect DMA is mangled on HW; one offset per
                # partition row (the unroll-supported form) is correct
                nc.gpsimd.indirect_dma_start(
                    out=g[:, b, :], out_offset=None,
                    in_=table_ap,
                    in_offset=IndirectOffsetOnAxis(
                        ap=srcit[:, col:col + 1], axis=0),
                )
                nc.gpsimd.indirect_dma_start(
                    out=a[:, b, :], out_offset=None,
                    in_=ad_ap,
                    in_offset=IndirectOffsetOnAxis(
                        ap=dstit[:, col:col + 1], axis=0),
                )
            oh = ohp.tile([128, NB, 128], F32, tag="oh")
            nc.vector.tensor_tensor(
                out=oh[:],
                in0=relt[:, w * NB:(w + 1) * NB].unsqueeze(2)
                    .broadcast_to([128, NB, 128]),
                in1=iotaf[:].rearrange("p (b j) -> p b j", j=128),
                op=mybir.AluOpType.is_equal,
            )
            s = sm.tile([128, NB, HD], F32, tag="s")
            nc.vector.tensor_tensor(
                out=s[:], in0=g[:, :, FD:TC], in1=a[:], op=mybir.AluOpType.add)
            lkn = sm.tile([128, NB, HD], F32, tag="lkn")
            nc.vector.tensor_scalar(
                out=lkn[:], in0=s[:], scalar1=0.0, scalar2=0.2,
                op0=mybir.AluOpType.min, op1=mybir.AluOpType.mult)
            lkp = sm.tile([128, NB, HD], F32, tag="lkp")
            nc.vector.tensor_scalar_max(lkp[:], s[:], 0.0)
            e = sm.tile([128, NB, HD], F32, tag="e")
            nc.vector.tensor_tensor(
                out=e[:], in0=lkn[:], in1=lkp[:], op=mybir.AluOpType.add)
            ex = sm.tile([128, NB, HD], F32, tag="ex")
            nc.scalar.activation(out=ex[:], in_=e[:],
                                 func=mybir.ActivationFunctionType.Exp)
            wm = wp.tile([128, NB, TC], F32, tag="wm")
            nc.vector.tensor_tensor(
                out=wm[:, :, 0:FD].rearrange("p b (h c) -> p b h c", h=HD),
                in0=g[:, :, 0:FD].rearrange("p b (h c) -> p b h c", h=HD),
                in1=ex[:].unsqueeze(3).broadcast_to([128, NB, HD, FD // HD]),
                op=mybir.AluOpType.mult,
            )
            nc.vector.tensor_copy(out=wm[:, :, FD:TC], in_=ex[:])
            psw = pw.tile([128, TC], F32, tag="psw")
            for b in range(NB):
                nc.tensor.matmul(
                    out=psw[:], lhsT=oh[:, b, :], rhs=wm[:, b, :],
                    start=(b == 0), stop=(b == NB - 1),
                )
            rec = sm.tile([128, HD], F32, tag="rec")
            nc.vector.reciprocal(rec[:nrows], psw[:nrows, FD:TC])
            res = sm.tile([128, FD], F32, tag="res")
            if nrows < 128:
                nc.vector.memset(res[:], 0.0)
            nc.vector.tensor_tensor(
                out=res[:nrows, :].rearrange("p (h c) -> p h c", h=HD),
                in0=psw[:nrows, 0:FD].rearrange("p (h c) -> p h c", h=HD),
                in1=rec[:nrows].unsqueeze(2).broadcast_to([nrows, HD, FD // HD]),
                op=mybir.AluOpType.mult,
            )
            sink(w, nrows, res)

    def l1_sink(w, nrows, res):
        # ELU -> transpose -> table2 row chunk [h2+b2 | as2 | ad2]
        mn = sm.tile([128, F1], F32, tag="mn")
        nc.vector.tensor_scalar_min(mn[:], res[:], 0.0)
        en = sm.tile([128, F1], F32, tag="en")
        nc.scalar.activation(out=en[:], in_=mn[:],
                             func=mybir.ActivationFunctionType.Exp)
        mp = sm.tile([128, F1], F32, tag="mp")
        nc.vector.tensor_scalar(
            out=mp[:], in0=res[:], scalar1=0.0, scalar2=-1.0,
            op0=mybir.AluOpType.max, op1=mybir.AluOpType.add)
        hp = sm.tile([128, F1], F32, tag="hp")
        nc.vector.tensor_tensor(
            out=hp[:], in0=en[:], in1=mp[:], op=mybir.AluOpType.add)
        # for the final window, res rows beyond nrows are 0, so hp rows are
        # exp(0)-1+max(0,0) = 0 there — no masking needed before the transpose
        pst = pt.tile([128, 128], F32)
        nc.tensor.transpose(out=pst[:], in_=hp[:], identity=ident[:])
        hpt = sm.tile([128, 128], F32, tag="hpt")
        nc.vector.tensor_copy(out=hpt[:], in_=pst[:])
        ps2 = p2.tile([128, F2 + 2 * H2], F32)
        nc.tensor.matmul(out=ps2[:], lhsT=hpt[:], rhs=w2t[:], start=True, stop=True)
        t2 = sm.tile([128, F2 + 2 * H2], F32, tag="t2")
        nc.vector.tensor_tensor(
            out=t2[:], in0=ps2[:], in1=b2t[:], op=mybir.AluOpType.add)
        nc.sync.dma_start(
            out=t2loc.ap()[w * 128:w * 128 + nrows, :], in_=t2[:nrows, 0:T2C])
        nc.sync.dma_start(
            out=ad2loc.ap()[w * 128:w * 128 + nrows, :],
            in_=t2[:nrows, T2C:T2C + H2])

    def l2_sink(w, nrows, res):
        nc.sync.dma_start(
            out=outs['out'][w * 128:w * 128 + nrows, :], in_=res[:nrows, :])

    edge_layer(table1.ap(), ad1.ap(), F1, H1, l1_sink)

    nc.gpsimd.collective_compute(
        "AllGather", mybir.AluOpType.bypass,
        replica_groups=[list(range(NCORES))],
        ins=[t2loc.ap().opt()], outs=[table2.ap().opt()],
    )
    nc.gpsimd.collective_compute(
        "AllGather", mybir.AluOpType.bypass,
        replica_groups=[list(range(NCORES))],
        ins=[ad2loc.ap().opt()], outs=[ad2.ap().opt()],
    )

    edge_layer(table2.ap(), ad2.ap(), F2, H2, l2_sink)


def prepare_host_inputs(x, edge_index, W1, a_src1, a_dst1, b1, W2, a_src2,
                        a_dst2, b2):
    structs, NB = _build_structures(np.asarray(edge_index))
    x = np.asarray(x, dtype=np.float32)
    xT = np.ascontiguousarray(x.T)
    v_s1 = np.einsum('ihc,hc->ih', np.asarray(W1).reshape(F1, H1, C1),
                     np.asarray(a_src1)).astype(np.float32)
    v_d1 = np.einsum('ihc,hc->ih', np.asarray(W1).reshape(F1, H1, C1),
                     np.asarray(a_dst1)).astype(np.float32)
    v_s2 = np.einsum('ihc,hc->ih', np.asarray(W2).reshape(F1, H2, C2),
                     np.asarray(a_src2)).astype(np.float32)
    v_d2 = np.einsum('ihc,hc->ih', np.asarray(W2).reshape(F1, H2, C2),
                     np.asarray(a_dst2)).astype(np.float32)
    w1ext = np.concatenate([np.asarray(W1, np.float32), v_s1, v_d1], axis=1)
    w2ext = np.concatenate([np.asarray(W2, np.float32), v_s2, v_d2], axis=1)
    bias1 = np.concatenate([np.asarray(b1, np.float32), np.zeros(2 * H1, np.float32)])
    bias2 = np.concatenate([np.asarray(b2, np.float32), np.zeros(2 * H2, np.float32)])
    bias1B = np.ascontiguousarray(np.broadcast_to(bias1, (128, F1 + 2 * H1)))
    bias2B = np.ascontiguousarray(np.broadcast_to(bias2, (128, F2 + 2 * H2)))
    shared = dict(xT=xT, w1ext=w1ext, w2ext=w2ext, bias1B=bias1B, bias2B=bias2B)
    in_maps = []
    for srcI, dstI, relX in structs:
        in_maps.append(dict(shared, srcI=srcI, dstI=dstI, relX=relX))
    return in_maps, NB


_PROGRAM_CACHE = {}


def _build_full_program(NB):
    if NB in _PROGRAM_CACHE:
        return _PROGRAM_CACHE[NB]
    nc = bacc.Bacc(trn_type="TRN2", num_devices=NCORES, debug=False)
    specs = {
        'xT': [128, N], 'w1ext': [128, F1 + 2 * H1], 'w2ext': [128, F2 + 2 * H2],
        'bias1B': [128, F1 + 2 * H1], 'bias2B': [128, F2 + 2 * H2],
    }
    ins = {}
    for name, shape in specs.items():
        ins[name] = nc.dram_tensor(name, shape, F32, kind="ExternalInput").ap()
    for name in ('srcI', 'dstI'):
        ins[name] = nc.dram_tensor(name, [128, NWIN * NB], I32,
                                   kind="ExternalInput").ap()
    ins['relX'] = nc.dram_tensor('relX', [128, NWIN * NB], F32,
                                 kind="ExternalInput").ap()
    outs = {'out': nc.dram_tensor('out', [NPC, F2], F32,
                                  kind="ExternalOutput").ap()}
    with tile.TileContext(nc) as tc:
        with ExitStack() as ctx:
            gat_program(ctx, tc, ins, outs, NB)
    nc.compile()
    _PROGRAM_CACHE[NB] = nc
    return nc


def kernel(**inputs) -> np.ndarray:
    in_maps, NB = prepare_host_inputs(**inputs)
    nc = _build_full_program(NB)
    res = run_bass_kernel_spmd(nc, in_maps, core_ids=list(range(NCORES)))
    return np.concatenate([r['out'] for r in res.results], axis=0)

